# revision 10
# baseline (speedup 1.0000x reference)
"""Trainium2 Bass kernel for nn_Attention_38130719654026 (sparse_attention).

The reference collapses exactly (verified to 9e-8 rel err): the top-k gather
broadcasts kv over the topk axis, so attention logits are constant along it,
softmax is uniform, and attn @ v_sel returns v unchanged.  grad/q/k are dead.
What remains is:

    g   = gelu(x @ W_v)              with W_v = W_qkv[:, 1024:1536]
    y[b, P, n, 64H:64H+64] = g[b, 2H + P//8, n, 64*(P%8) : 64*(P%8)+64]
    out = y @ W_out + b_out

Sharding (8 cores, no collectives): core c -> (batch b = c//4,
window parity q = (c//2)%2, token half t = c%2).  Each core computes the 8
output windows P in [8q, 8q+8) for its 128-token slice; these need exactly
the 8 source windows p = q + 2H at the same tokens, so x is read exactly
once across the fleet.

On-chip per core (bf16 compute, fp32 accumulate):
  mm1: gT(512 ch, 1024 tok) = W_v^T @ X^T,  X^T pre-transposed on host.
  gelu (erf) on ScalarE, PSUM -> SBUF bf16.
  mm2 per window P: out = y_P @ W_out as 8 accumulating K=64 matmuls whose
  lhsT is a 64-partition slice of gT; rhs comes from host-duplicated W_out
  row-tiles so lhsT/rhs partition bases match (0 or 64 by window parity).

All inputs arrive as ONE host-packed DRAM tensor in exact SBUF layout
(single fully-contiguous DMA; also keeps the walrus per-instruction sync
wait count under the ISA limit), output leaves as one DMA.
"""

import sys

sys.path.insert(0, "/opt/trn_rl_repo")

import numpy as np
import ml_dtypes

B, P_WIN, N_TOK, DIM = 2, 16, 256, 512
H_HEADS, DH = 8, 64
INNER = H_HEADS * DH  # 512
TOK_HALF = N_TOK // 2  # 128
N_CORES = 8

# Column layout of the packed input mega-tile (128 partitions, bf16):
#   [0:4096)      xt   : X^T as 4 chunks of (128, 1024)
#   [4096:6144)   wv   : W_v as 4 chunks of (128, 512)
#   [6144:10240)  wo   : W_out as 8 dup-row tiles of (128, 512)
XT_OFF, WV_OFF, WO_OFF, IN_COLS = 0, 4096, 6144, 10240

_COMPILED = None


def _build_bass():
    """Raw bass (no TileContext): this walrus build rejects instructions
    carrying more than one embedded sync wait, which Tile's kernel-tail
    drain always produces.  Manual blocks emit every wait as its own
    instruction.  Engine plan: sync = 2 DMAs, PE = 96 matmuls,
    ACT = 8 gelu + 8 PSUM->SBUF copies."""
    import concourse.bass as bass
    import concourse.mybir as mybir

    dt = mybir.dt
    nc = bass.Bass()

    inp = nc.declare_dram_parameter("inp", [128, IN_COLS], dt.bfloat16, isOutput=False)
    out = nc.declare_dram_parameter("out", [8, 128, 512], dt.float32, isOutput=True)

    with (
        nc.sbuf_tensor([128, IN_COLS], dt.bfloat16) as mega,
        nc.sbuf_tensor([128, 4, 1024], dt.bfloat16) as g_t,
        nc.sbuf_tensor([128, 8, 512], dt.float32) as out_t,
        nc.psum_tensor([128, 4, 512], dt.float32) as ps1,
        nc.psum_tensor([128, 4, 512], dt.float32) as ps2,
        nc.semaphore("dma_sem") as dma_sem,
        nc.semaphore("pe1_sem") as pe1_sem,
        nc.semaphore("pe2_sem") as pe2_sem,
        nc.semaphore("act_sem") as act_sem,
        nc.Block() as block,
    ):

        def xt_sl(k, c0, n):
            return mega[:, XT_OFF + 1024 * k + c0 : XT_OFF + 1024 * k + c0 + n]

        def wv_sl(k, m):
            return mega[:, WV_OFF + 512 * k + 128 * m : WV_OFF + 512 * k + 128 * m + 128]

        def wo_sl(h, jp):
            return mega[jp : jp + 64, WO_OFF + 512 * h : WO_OFF + 512 * h + 512]

        @block.sync
        def _(sync):
            sync.dma_start(out=mega[:], in_=inp[:]).then_inc(dma_sem, 16)
            sync.wait_ge(act_sem, 16)
            sync.dma_start(
                out=out[:].rearrange("h p t -> p h t"), in_=out_t[:]
            ).then_inc(dma_sem, 16)
            sync.wait_ge(dma_sem, 32)

        @block.tensor
        def _(tensor):
            tensor.wait_ge(dma_sem, 16)
            # mm1: group i = 2*m + nch computes gT chan-tile m, token chunk nch
            for i in range(8):
                m, nch = i // 2, i % 2
                if i >= 4:
                    tensor.wait_ge(act_sem, i - 3)  # bank i%4 freed by gelu i-4
                for k in range(4):
                    mm = nc.tensor.matmul(
                        ps1[:, i % 4, :],
                        lhsT=wv_sl(k, m),
                        rhs=xt_sl(k, 512 * nch, 512),
                        start=(k == 0),
                        stop=(k == 3),
                    )
                    if k == 3:
                        mm.then_inc(pe1_sem, 1)
            # mm2: window pl; lhsT = 64-row slice of g tile jt, parity jp
            for pl in range(8):
                jt, jp = pl // 2, 64 * (pl % 2)
                need = 2 * jt + 2  # gelu groups 2jt, 2jt+1 written
                if pl >= 4:
                    need = max(need, pl + 5)  # bank pl%4 freed by copy pl-4
                tensor.wait_ge(act_sem, need)
                for hh in range(8):
                    mm = nc.tensor.matmul(
                        ps2[:, pl % 4, :],
                        lhsT=g_t[jp : jp + 64, jt, 128 * hh : 128 * hh + 128],
                        rhs=wo_sl(hh, jp),
                        start=(hh == 0),
                        stop=(hh == 7),
                    )
                    if hh == 7:
                        mm.then_inc(pe2_sem, 1)

        @block.scalar
        def _(scalar):
            for i in range(8):
                m, nch = i // 2, i % 2
                scalar.wait_ge(pe1_sem, i + 1)
                nc.scalar.activation(
                    g_t[:, m, 512 * nch : 512 * nch + 512],
                    ps1[:, i % 4, :],
                    mybir.ActivationFunctionType.Gelu,
                ).then_inc(act_sem, 1)
            for pl in range(8):
                scalar.wait_ge(pe2_sem, pl + 1)
                nc.scalar.copy(out_t[:, pl, :], ps2[:, pl % 4, :]).then_inc(act_sem, 1)

    return nc


def _shard_inputs(x, W_qkv, W_out):
    bf16 = ml_dtypes.bfloat16
    W_v = np.ascontiguousarray(W_qkv[:, 2 * INNER : 3 * INNER]).astype(bf16)
    wv_part = W_v.reshape(4, 128, 512).transpose(1, 0, 2).reshape(128, 2048)
    # wo[H] = W_out rows [64H, 64H+64) duplicated to both partition halves.
    wo = np.empty((8, 128, 512), dtype=bf16)
    for h in range(8):
        blk = W_out[64 * h : 64 * h + 64, :].astype(bf16)
        wo[h, :64] = blk
        wo[h, 64:] = blk
    wo_part = wo.transpose(1, 0, 2).reshape(128, 4096)
    in_maps = []
    for c in range(N_CORES):
        b, q, t = c // 4, (c // 2) % 2, c % 2
        xs = x[b, q::2, TOK_HALF * t : TOK_HALF * (t + 1), :]  # (8, 128, 512)
        xt = np.ascontiguousarray(xs.transpose(2, 0, 1).reshape(512, 1024)).astype(bf16)
        xt_part = xt.reshape(4, 128, 1024).transpose(1, 0, 2).reshape(128, 4096)
        mega = np.concatenate([xt_part, wv_part, wo_part], axis=1)
        in_maps.append({"inp": np.ascontiguousarray(mega)})
    return in_maps


def _assemble(results, b_out):
    out = np.empty((B, P_WIN, N_TOK, DIM), dtype=np.float32)
    for c in range(N_CORES):
        b, q, t = c // 4, (c // 2) % 2, c % 2
        r = results[c]["out"]  # (8, 128, 512)
        out[b, 8 * q : 8 * q + 8, TOK_HALF * t : TOK_HALF * (t + 1), :] = r
    out += b_out.astype(np.float32)
    return out


def _run(inputs, trace=False, trace_cores=None):
    global _COMPILED
    from concourse.bass_utils import run_bass_kernel_spmd

    if _COMPILED is None:
        _COMPILED = _build_bass()
    nc = _COMPILED
    in_maps = _shard_inputs(
        np.asarray(inputs["x"]), np.asarray(inputs["W_qkv"]), np.asarray(inputs["W_out"])
    )
    res = run_bass_kernel_spmd(
        nc, in_maps, core_ids=list(range(N_CORES)), trace=trace, trace_cores=trace_cores
    )
    out = _assemble(res.results, np.asarray(inputs["b_out"]))
    return out, res


def kernel(x, grad, W_qkv, W_out, b_out):
    out, _ = _run(dict(x=x, grad=grad, W_qkv=W_qkv, W_out=W_out, b_out=b_out))
    return out


# revision 12
# speedup vs baseline: 1.4249x; 1.4249x over previous
"""Trainium2 Bass kernel for nn_Attention_38130719654026 (sparse_attention).

The reference collapses exactly (verified to 9e-8 rel err): the top-k gather
broadcasts kv over the topk axis, so attention logits are constant along it,
softmax is uniform, and attn @ v_sel returns v unchanged.  grad/q/k are dead.
What remains is:

    g   = gelu(x @ W_v)              with W_v = W_qkv[:, 1024:1536]
    y[b, P, n, 64H:64H+64] = g[b, 2H + P//8, n, 64*(P%8) : 64*(P%8)+64]
    out = y @ W_out + b_out

Sharding (8 cores, no collectives): core c -> (batch b = c//4,
window parity q = (c//2)%2, token half t = c%2).  Each core computes the 8
output windows P in [8q, 8q+8) for its 128-token slice; these need exactly
the 8 source windows p = q + 2H at the same tokens, so x is read exactly
once across the fleet.

Raw bass (no TileContext: this walrus build rejects >1 embedded sync wait
per instruction, which Tile's kernel-tail drain always produces).  bf16
compute, fp32 PSUM accumulate, fp32 output.

Pipeline per core:
  DMA1 (wv + x^T token-half A) -> mm1 nch=0 | DMA2 (x^T half B) -> mm1
  nch=1 | DMA3 (W_out dup tiles) -> mm2.  Gelu on ScalarE (table
  pre-warmed), PSUM->SBUF copies on DVE, 4 progressive output DMAs.
  mm2 emits window pairs interleaved: adjacent matmuls use PE row groups
  0-1 vs 2-3 (partition base 0 vs 64) and different PSUM banks, so they
  run concurrently.
"""

import sys

sys.path.insert(0, "/opt/trn_rl_repo")

import numpy as np
import ml_dtypes

B, P_WIN, N_TOK, DIM = 2, 16, 256, 512
H_HEADS, DH = 8, 64
INNER = H_HEADS * DH  # 512
TOK_HALF = N_TOK // 2  # 128
N_CORES = 8

# Column layout of the packed input mega-tile (128 partitions, bf16):
#   [0:2048)      wv  : W_v as 4 chunks of (128, 512)
#   [2048:4096)   xtA : X^T token-half A, 4 chunks of (128, 512)
#   [4096:6144)   xtB : X^T token-half B, 4 chunks of (128, 512)
#   [6144:10240)  wo  : W_out as 8 dup-row tiles of (128, 512)
WV_OFF, XTA_OFF, XTB_OFF, WO_OFF, IN_COLS = 0, 2048, 4096, 6144, 10240

_COMPILED = None


def _build_bass():
    import concourse.bass as bass
    import concourse.mybir as mybir

    dt = mybir.dt
    nc = bass.Bass()

    inp = nc.declare_dram_parameter("inp", [128, IN_COLS], dt.bfloat16, isOutput=False)
    out = nc.declare_dram_parameter("out", [8, 128, 512], dt.float32, isOutput=True)

    with (
        nc.sbuf_tensor([128, IN_COLS], dt.bfloat16) as mega,
        nc.sbuf_tensor([128, 4, 1024], dt.bfloat16) as g_t,
        nc.sbuf_tensor([128, 8, 512], dt.float32) as out_t,
        nc.sbuf_tensor([128, 16], dt.float32) as scratch,
        nc.psum_tensor([128, 4, 512], dt.float32) as ps1,
        nc.psum_tensor([128, 4, 512], dt.float32) as ps2,
        nc.semaphore("dma1_sem") as dma1_sem,
        nc.semaphore("dma2_sem") as dma2_sem,
        nc.semaphore("dma3_sem") as dma3_sem,
        nc.semaphore("dmao_sem") as dmao_sem,
        nc.semaphore("pe1_sem") as pe1_sem,
        nc.semaphore("pe2_sem") as pe2_sem,
        nc.semaphore("act_sem") as act_sem,
        nc.semaphore("dve_sem") as dve_sem,
        nc.Block() as block,
    ):
        # group i of mm1: i = 0..7 -> (m = i%4, nch = i//4); PSUM bank i%4.
        def xt_sl(k, nch):
            off = (XTA_OFF, XTB_OFF)[nch] + 512 * k
            return mega[:, off : off + 512]

        def wv_sl(k, m):
            return mega[:, WV_OFF + 512 * k + 128 * m : WV_OFF + 512 * k + 128 * m + 128]

        def wo_sl(h, jp):
            return mega[jp : jp + 64, WO_OFF + 512 * h : WO_OFF + 512 * h + 512]

        @block.sync
        def _(sync):
            sync.dma_start(out=mega[:, :XTB_OFF], in_=inp[:, :XTB_OFF]).then_inc(dma1_sem, 16)
            sync.dma_start(out=mega[:, XTB_OFF:WO_OFF], in_=inp[:, XTB_OFF:WO_OFF]).then_inc(dma2_sem, 16)
            sync.dma_start(out=mega[:, WO_OFF:], in_=inp[:, WO_OFF:]).then_inc(dma3_sem, 16)
            for pp in range(4):
                sync.wait_ge(dve_sem, 2 * pp + 3)
                sync.dma_start(
                    out=out[2 * pp : 2 * pp + 2].rearrange("h p t -> p h t"),
                    in_=out_t[:, 2 * pp : 2 * pp + 2, :],
                ).then_inc(dmao_sem, 16)
            sync.wait_ge(dmao_sem, 64)

        @block.tensor
        def _(tensor):
            # mm1: gT = W_v^T @ X^T (8 groups of 4 accumulating K=128 matmuls)
            for i in range(8):
                m, nch = i % 4, i // 4
                if i == 0:
                    tensor.wait_ge(dma1_sem, 16)  # DMA1: wv + xtA
                if i == 4:
                    tensor.wait_ge(dma2_sem, 16)  # DMA2: xtB
                if i >= 4:
                    tensor.wait_ge(act_sem, i - 3)  # bank i%4 freed by gelu i-4
                for k in range(4):
                    mm = nc.tensor.matmul(
                        ps1[:, i % 4, :],
                        lhsT=wv_sl(k, m),
                        rhs=xt_sl(k, nch),
                        start=(k == 0),
                        stop=(k == 3),
                    )
                    if k == 3:
                        mm.then_inc(pe1_sem, 1)
            # mm2: window pairs (2pp, 2pp+1) interleaved across PE row groups
            tensor.wait_ge(dma3_sem, 16)  # DMA3: wo
            for pp in range(4):
                tensor.wait_ge(act_sem, 5 + pp)  # g tile pp complete
                if pp >= 2:
                    tensor.wait_ge(dve_sem, 2 * pp - 1)  # banks freed by copies
                for hh in range(8):
                    for pl in (2 * pp, 2 * pp + 1):
                        jp = 64 * (pl % 2)
                        mm = nc.tensor.matmul(
                            ps2[:, pl % 4, :],
                            lhsT=g_t[jp : jp + 64, pp, 128 * hh : 128 * hh + 128],
                            rhs=wo_sl(hh, jp),
                            start=(hh == 0),
                            stop=(hh == 7),
                            skip_group_check=True,
                        )
                        if hh == 7:
                            mm.then_inc(pe2_sem, 1)

        @block.scalar
        def _(scalar):
            # Pre-warm the gelu spline table during the input DMA.
            scalar.wait_ge(dve_sem, 1)
            nc.scalar.activation(
                scratch[:, 8:], scratch[:, :8], mybir.ActivationFunctionType.Gelu
            )
            for i in range(8):
                m, nch = i % 4, i // 4
                scalar.wait_ge(pe1_sem, i + 1)
                nc.scalar.activation(
                    g_t[:, m, 512 * nch : 512 * nch + 512],
                    ps1[:, i % 4, :],
                    mybir.ActivationFunctionType.Gelu,
                ).then_inc(act_sem, 1)

        @block.vector
        def _(vector):
            nc.vector.memset(scratch[:, :8], 0.0).then_inc(dve_sem, 1)
            for pl in range(8):
                vector.wait_ge(pe2_sem, pl + 1)
                nc.vector.tensor_copy(out_t[:, pl, :], ps2[:, pl % 4, :]).then_inc(
                    dve_sem, 1
                )

    return nc


def _shard_inputs(x, W_qkv, W_out):
    bf16 = ml_dtypes.bfloat16
    W_v = np.ascontiguousarray(W_qkv[:, 2 * INNER : 3 * INNER]).astype(bf16)
    wv_part = W_v.reshape(4, 128, 512).transpose(1, 0, 2).reshape(128, 2048)
    # wo[H] = W_out rows [64H, 64H+64) duplicated to both partition halves.
    wo = np.empty((8, 128, 512), dtype=bf16)
    for h in range(8):
        blk = W_out[64 * h : 64 * h + 64, :].astype(bf16)
        wo[h, :64] = blk
        wo[h, 64:] = blk
    wo_part = wo.transpose(1, 0, 2).reshape(128, 4096)
    in_maps = []
    for c in range(N_CORES):
        b, q, t = c // 4, (c // 2) % 2, c % 2
        xs = x[b, q::2, TOK_HALF * t : TOK_HALF * (t + 1), :]  # (8, 128, 512)
        xt = np.ascontiguousarray(xs.transpose(2, 0, 1).reshape(512, 1024)).astype(bf16)
        # split into token halves A (first 512 of the 1024 = windows 0..3)
        # NOTE: xt free dim is (window s 0..7) x (token 0..127); halves are
        # windows 0-3 vs 4-7 -> matches mm1 nch chunking of 512 tokens.
        xt4 = xt.reshape(4, 128, 1024)
        xtA = xt4[:, :, :512].transpose(1, 0, 2).reshape(128, 2048)
        xtB = xt4[:, :, 512:].transpose(1, 0, 2).reshape(128, 2048)
        mega = np.concatenate([wv_part, xtA, xtB, wo_part], axis=1)
        in_maps.append({"inp": np.ascontiguousarray(mega)})
    return in_maps


def _assemble(results, b_out):
    out = np.empty((B, P_WIN, N_TOK, DIM), dtype=np.float32)
    for c in range(N_CORES):
        b, q, t = c // 4, (c // 2) % 2, c % 2
        r = results[c]["out"]  # (8, 128, 512)
        out[b, 8 * q : 8 * q + 8, TOK_HALF * t : TOK_HALF * (t + 1), :] = r
    out += b_out.astype(np.float32)
    return out


def _run(inputs, trace=False, trace_cores=None):
    global _COMPILED
    from concourse.bass_utils import run_bass_kernel_spmd

    if _COMPILED is None:
        _COMPILED = _build_bass()
    nc = _COMPILED
    in_maps = _shard_inputs(
        np.asarray(inputs["x"]), np.asarray(inputs["W_qkv"]), np.asarray(inputs["W_out"])
    )
    res = run_bass_kernel_spmd(
        nc, in_maps, core_ids=list(range(N_CORES)), trace=trace, trace_cores=trace_cores
    )
    out = _assemble(res.results, np.asarray(inputs["b_out"]))
    return out, res


def kernel(x, grad, W_qkv, W_out, b_out):
    out, _ = _run(dict(x=x, grad=grad, W_qkv=W_qkv, W_out=W_out, b_out=b_out))
    return out


# revision 13
# speedup vs baseline: 1.4672x; 1.0297x over previous
"""Trainium2 Bass kernel for nn_Attention_38130719654026 (sparse_attention).

The reference collapses exactly (verified to 9e-8 rel err): the top-k gather
broadcasts kv over the topk axis, so attention logits are constant along it,
softmax is uniform, and attn @ v_sel returns v unchanged.  grad/q/k are dead.
What remains is:

    g   = gelu(x @ W_v)              with W_v = W_qkv[:, 1024:1536]
    y[b, P, n, 64H:64H+64] = g[b, 2H + P//8, n, 64*(P%8) : 64*(P%8)+64]
    out = y @ W_out + b_out

Sharding (8 cores, no collectives): core c -> (batch b = c//4,
window parity q = (c//2)%2, token half t = c%2).  Each core computes the 8
output windows P in [8q, 8q+8) for its 128-token slice; these need exactly
the 8 source windows p = q + 2H at the same tokens, so x is read exactly
once across the fleet.

Raw bass (no TileContext: this walrus build rejects >1 embedded sync wait
per instruction, which Tile's kernel-tail drain always produces).  bf16
compute, fp32 PSUM accumulate, bf16 output (host casts back to fp32).

Pipeline per core (one PSUM bank per matmul group, all 8 banks used):
  7 input DMAs gate compute progressively: wv -> xtA k-chunks (k-major
  mm1 starts after the first 0.625 MB) -> xtB -> wo.  Gelu on ScalarE
  (spline table pre-warmed during the DMA), PSUM->SBUF copies on DVE,
  4 progressive output DMAs.  mm2 emits window pairs interleaved:
  adjacent matmuls use PE row groups 0-1 vs 2-3 (partition base 0 vs 64)
  and different PSUM banks, so they run concurrently.
"""

import sys

sys.path.insert(0, "/opt/trn_rl_repo")

import numpy as np
import ml_dtypes

B, P_WIN, N_TOK, DIM = 2, 16, 256, 512
H_HEADS, DH = 8, 64
INNER = H_HEADS * DH  # 512
TOK_HALF = N_TOK // 2  # 128
N_CORES = 8

# Column layout of the packed input mega-tile (128 partitions, bf16):
#   [0:2048)      wv  : W_v as 4 chunks of (128, 512)
#   [2048:4096)   xtA : X^T token-half A, 4 k-chunks of (128, 512)
#   [4096:6144)   xtB : X^T token-half B, 4 k-chunks of (128, 512)
#   [6144:10240)  wo  : W_out as 8 dup-row tiles of (128, 512)
WV_OFF, XTA_OFF, XTB_OFF, WO_OFF, IN_COLS = 0, 2048, 4096, 6144, 10240

_COMPILED = None


def _build_bass():
    import concourse.bass as bass
    import concourse.mybir as mybir

    dt = mybir.dt
    nc = bass.Bass()

    inp = nc.declare_dram_parameter("inp", [128, IN_COLS], dt.bfloat16, isOutput=False)
    out = nc.declare_dram_parameter("out", [8, 128, 512], dt.bfloat16, isOutput=True)

    with (
        nc.sbuf_tensor([128, IN_COLS], dt.bfloat16) as mega,
        nc.sbuf_tensor([128, 4, 1024], dt.bfloat16) as g_t,
        nc.sbuf_tensor([128, 8, 512], dt.bfloat16) as out_t,
        nc.sbuf_tensor([128, 16], dt.float32) as scratch,
        nc.psum_tensor([128, 8, 512], dt.float32) as ps,
        nc.semaphore("wv_sem") as wv_sem,
        nc.semaphore("ka0_sem") as ka0_sem,
        nc.semaphore("ka1_sem") as ka1_sem,
        nc.semaphore("ka2_sem") as ka2_sem,
        nc.semaphore("ka3_sem") as ka3_sem,
        nc.semaphore("xb_sem") as xb_sem,
        nc.semaphore("wo_sem") as wo_sem,
        nc.semaphore("dmao_sem") as dmao_sem,
        nc.semaphore("pe1_sem") as pe1_sem,
        nc.semaphore("pe2_sem") as pe2_sem,
        nc.semaphore("act_sem") as act_sem,
        nc.semaphore("dve_sem") as dve_sem,
        nc.Block() as block,
    ):
        ka_sems = [ka0_sem, ka1_sem, ka2_sem, ka3_sem]

        def xt_sl(k, nch):
            off = (XTA_OFF, XTB_OFF)[nch] + 512 * k
            return mega[:, off : off + 512]

        def wv_sl(k, m):
            return mega[:, WV_OFF + 512 * k + 128 * m : WV_OFF + 512 * k + 128 * m + 128]

        def wo_sl(h, jp):
            return mega[jp : jp + 64, WO_OFF + 512 * h : WO_OFF + 512 * h + 512]

        @block.sync
        def _(sync):
            sync.dma_start(out=mega[:, :XTA_OFF], in_=inp[:, :XTA_OFF]).then_inc(wv_sem, 16)
            for k in range(4):
                o = XTA_OFF + 512 * k
                sync.dma_start(out=mega[:, o : o + 512], in_=inp[:, o : o + 512]).then_inc(
                    ka_sems[k], 16
                )
            sync.dma_start(out=mega[:, XTB_OFF:WO_OFF], in_=inp[:, XTB_OFF:WO_OFF]).then_inc(xb_sem, 16)
            sync.dma_start(out=mega[:, WO_OFF:], in_=inp[:, WO_OFF:]).then_inc(wo_sem, 16)
            for pp in range(4):
                sync.wait_ge(dve_sem, 2 * pp + 3)
                sync.dma_start(
                    out=out[2 * pp : 2 * pp + 2].rearrange("h p t -> p h t"),
                    in_=out_t[:, 2 * pp : 2 * pp + 2, :],
                ).then_inc(dmao_sem, 16)
            sync.wait_ge(dmao_sem, 64)

        @block.tensor
        def _(tensor):
            # mm1 (k-major): gT = W_v^T @ X^T; group (m, nch) -> bank 4*nch+m
            tensor.wait_ge(wv_sem, 16)
            for nch in range(2):
                for k in range(4):
                    if nch == 0:
                        tensor.wait_ge(ka_sems[k], 16)
                    elif k == 0:
                        tensor.wait_ge(xb_sem, 16)
                    for m in range(4):
                        mm = nc.tensor.matmul(
                            ps[:, 4 * nch + m, :],
                            lhsT=wv_sl(k, m),
                            rhs=xt_sl(k, nch),
                            start=(k == 0),
                            stop=(k == 3),
                            skip_group_check=True,
                        )
                        if k == 3:
                            mm.then_inc(pe1_sem, 1)
            # mm2: window pairs (2pp, 2pp+1) interleaved across PE row groups;
            # window pl -> bank pl (freed by gelu pl before act_sem >= 5+pp)
            tensor.wait_ge(wo_sem, 16)
            for pp in range(4):
                tensor.wait_ge(act_sem, 5 + pp)  # g tile pp complete
                for hh in range(8):
                    for pl in (2 * pp, 2 * pp + 1):
                        jp = 64 * (pl % 2)
                        mm = nc.tensor.matmul(
                            ps[:, pl, :],
                            lhsT=g_t[jp : jp + 64, pp, 128 * hh : 128 * hh + 128],
                            rhs=wo_sl(hh, jp),
                            start=(hh == 0),
                            stop=(hh == 7),
                            skip_group_check=True,
                        )
                        if hh == 7:
                            mm.then_inc(pe2_sem, 1)

        @block.scalar
        def _(scalar):
            # Pre-warm the gelu spline table during the input DMA.
            scalar.wait_ge(dve_sem, 1)
            nc.scalar.activation(
                scratch[:, 8:], scratch[:, :8], mybir.ActivationFunctionType.Gelu
            )
            for i in range(8):
                m, nch = i % 4, i // 4
                scalar.wait_ge(pe1_sem, i + 1)
                nc.scalar.activation(
                    g_t[:, m, 512 * nch : 512 * nch + 512],
                    ps[:, 4 * nch + m, :],
                    mybir.ActivationFunctionType.Gelu,
                ).then_inc(act_sem, 1)

        @block.vector
        def _(vector):
            nc.vector.memset(scratch[:, :8], 0.0).then_inc(dve_sem, 1)
            for pl in range(8):
                vector.wait_ge(pe2_sem, pl + 1)
                nc.vector.tensor_copy(out_t[:, pl, :], ps[:, pl, :]).then_inc(
                    dve_sem, 1
                )

    return nc


def _shard_inputs(x, W_qkv, W_out):
    bf16 = ml_dtypes.bfloat16
    W_v = np.ascontiguousarray(W_qkv[:, 2 * INNER : 3 * INNER]).astype(bf16)
    wv_part = W_v.reshape(4, 128, 512).transpose(1, 0, 2).reshape(128, 2048)
    # wo[H] = W_out rows [64H, 64H+64) duplicated to both partition halves.
    wo = np.empty((8, 128, 512), dtype=bf16)
    for h in range(8):
        blk = W_out[64 * h : 64 * h + 64, :].astype(bf16)
        wo[h, :64] = blk
        wo[h, 64:] = blk
    wo_part = wo.transpose(1, 0, 2).reshape(128, 4096)
    in_maps = []
    for c in range(N_CORES):
        b, q, t = c // 4, (c // 2) % 2, c % 2
        xs = x[b, q::2, TOK_HALF * t : TOK_HALF * (t + 1), :]  # (8, 128, 512)
        xt = np.ascontiguousarray(xs.transpose(2, 0, 1).reshape(512, 1024)).astype(bf16)
        xt4 = xt.reshape(4, 128, 1024)
        xtA = xt4[:, :, :512].transpose(1, 0, 2).reshape(128, 2048)
        xtB = xt4[:, :, 512:].transpose(1, 0, 2).reshape(128, 2048)
        mega = np.concatenate([wv_part, xtA, xtB, wo_part], axis=1)
        in_maps.append({"inp": np.ascontiguousarray(mega)})
    return in_maps


def _assemble(results, b_out):
    out = np.empty((B, P_WIN, N_TOK, DIM), dtype=np.float32)
    for c in range(N_CORES):
        b, q, t = c // 4, (c // 2) % 2, c % 2
        r = np.asarray(results[c]["out"]).astype(np.float32)  # (8, 128, 512)
        out[b, 8 * q : 8 * q + 8, TOK_HALF * t : TOK_HALF * (t + 1), :] = r
    out += b_out.astype(np.float32)
    return out


def _run(inputs, trace=False, trace_cores=None):
    global _COMPILED
    from concourse.bass_utils import run_bass_kernel_spmd

    if _COMPILED is None:
        _COMPILED = _build_bass()
    nc = _COMPILED
    in_maps = _shard_inputs(
        np.asarray(inputs["x"]), np.asarray(inputs["W_qkv"]), np.asarray(inputs["W_out"])
    )
    res = run_bass_kernel_spmd(
        nc, in_maps, core_ids=list(range(N_CORES)), trace=trace, trace_cores=trace_cores
    )
    out = _assemble(res.results, np.asarray(inputs["b_out"]))
    return out, res


def kernel(x, grad, W_qkv, W_out, b_out):
    out, _ = _run(dict(x=x, grad=grad, W_qkv=W_qkv, W_out=W_out, b_out=b_out))
    return out


# revision 14
# speedup vs baseline: 1.5398x; 1.0495x over previous
"""Trainium2 Bass kernel for nn_Attention_38130719654026 (sparse_attention).

The reference collapses exactly (verified to 9e-8 rel err): the top-k gather
broadcasts kv over the topk axis, so attention logits are constant along it,
softmax is uniform, and attn @ v_sel returns v unchanged.  grad/q/k are dead.
What remains is:

    g   = gelu(x @ W_v)              with W_v = W_qkv[:, 1024:1536]
    y[b, P, n, 64H:64H+64] = g[b, 2H + P//8, n, 64*(P%8) : 64*(P%8)+64]
    out = y @ W_out + b_out

Sharding (8 cores, no collectives): core c -> (batch b = c//4,
window parity q = (c//2)%2, token half t = c%2).  Each core computes the 8
output windows P in [8q, 8q+8) for its 128-token slice; these need exactly
the 8 source windows p = q + 2H at the same tokens, so x is read exactly
once across the fleet.

Raw bass (no TileContext: this walrus build rejects >1 embedded sync wait
per instruction, which Tile's kernel-tail drain always produces).  bf16
compute, fp32 PSUM accumulate, bf16 output (host casts back to fp32).

Pipeline per core (one PSUM bank per matmul group, all 8 banks used):
  7 input DMAs gate compute progressively: wv -> xtA k-chunks (k-major
  mm1 starts after the first 0.625 MB) -> xtB -> wo.  Gelu on ScalarE
  (spline table pre-warmed during the DMA), PSUM->SBUF copies on DVE,
  4 progressive output DMAs.  mm2 emits window pairs interleaved:
  adjacent matmuls use PE row groups 0-1 vs 2-3 (partition base 0 vs 64)
  and different PSUM banks, so they run concurrently.
"""

import sys

sys.path.insert(0, "/opt/trn_rl_repo")

import numpy as np
import ml_dtypes

B, P_WIN, N_TOK, DIM = 2, 16, 256, 512
H_HEADS, DH = 8, 64
INNER = H_HEADS * DH  # 512
TOK_HALF = N_TOK // 2  # 128
N_CORES = 8

# Column layout of the packed input mega-tile (128 partitions, bf16):
#   [1024k : 1024k+512)    wv k-chunk;  [1024k+512 : 1024k+1024) xtA k-chunk
#   [4096:6144)   xtB : X^T token-half B, 4 k-chunks of (128, 512)
#   [6144:10240)  wo  : W_out as 8 dup-row tiles of (128, 512)
XTB_OFF, WO_OFF, IN_COLS = 4096, 6144, 10240

_COMPILED = None


def _build_bass():
    import concourse.bass as bass
    import concourse.mybir as mybir

    dt = mybir.dt
    nc = bass.Bass()

    inp = nc.declare_dram_parameter("inp", [128, IN_COLS], dt.bfloat16, isOutput=False)
    out = nc.declare_dram_parameter("out", [8, 128, 512], dt.bfloat16, isOutput=True)

    with (
        nc.sbuf_tensor([128, IN_COLS], dt.bfloat16) as mega,
        nc.sbuf_tensor([128, 4, 1024], dt.bfloat16) as g_t,
        nc.sbuf_tensor([128, 8, 512], dt.bfloat16) as out_t,
        nc.sbuf_tensor([128, 16], dt.float32) as scratch,
        nc.psum_tensor([128, 8, 512], dt.float32) as ps,
        nc.semaphore("ka0_sem") as ka0_sem,
        nc.semaphore("ka1_sem") as ka1_sem,
        nc.semaphore("ka2_sem") as ka2_sem,
        nc.semaphore("ka3_sem") as ka3_sem,
        nc.semaphore("xb_sem") as xb_sem,
        nc.semaphore("wo_sem") as wo_sem,
        nc.semaphore("dmao_sem") as dmao_sem,
        nc.semaphore("pe1_sem") as pe1_sem,
        nc.semaphore("pe2_sem") as pe2_sem,
        nc.semaphore("act_sem") as act_sem,
        nc.semaphore("dve_sem") as dve_sem,
        nc.Block() as block,
    ):
        ka_sems = [ka0_sem, ka1_sem, ka2_sem, ka3_sem]

        def xt_sl(k, nch):
            off = (1024 * k + 512) if nch == 0 else (XTB_OFF + 512 * k)
            return mega[:, off : off + 512]

        def wv_sl(k, m):
            return mega[:, 1024 * k + 128 * m : 1024 * k + 128 * m + 128]

        def wo_sl(h, jp):
            return mega[jp : jp + 64, WO_OFF + 512 * h : WO_OFF + 512 * h + 512]

        @block.sync
        def _(sync):
            for k in range(4):
                o = 1024 * k
                sync.dma_start(out=mega[:, o : o + 1024], in_=inp[:, o : o + 1024]).then_inc(
                    ka_sems[k], 16
                )
            sync.dma_start(out=mega[:, XTB_OFF:WO_OFF], in_=inp[:, XTB_OFF:WO_OFF]).then_inc(xb_sem, 16)
            sync.dma_start(out=mega[:, WO_OFF:], in_=inp[:, WO_OFF:]).then_inc(wo_sem, 16)
            for pp in range(4):
                sync.wait_ge(dve_sem, 2 * pp + 3)
                sync.dma_start(
                    out=out[2 * pp : 2 * pp + 2].rearrange("h p t -> p h t"),
                    in_=out_t[:, 2 * pp : 2 * pp + 2, :],
                ).then_inc(dmao_sem, 16)
            sync.wait_ge(dmao_sem, 64)

        @block.tensor
        def _(tensor):
            # mm1 (k-major): gT = W_v^T @ X^T; group (m, nch) -> bank 4*nch+m
            for nch in range(2):
                for k in range(4):
                    if nch == 0:
                        tensor.wait_ge(ka_sems[k], 16)
                    elif k == 0:
                        tensor.wait_ge(xb_sem, 16)
                    for m in range(4):
                        mm = nc.tensor.matmul(
                            ps[:, 4 * nch + m, :],
                            lhsT=wv_sl(k, m),
                            rhs=xt_sl(k, nch),
                            start=(k == 0),
                            stop=(k == 3),
                            skip_group_check=True,
                        )
                        if k == 3:
                            mm.then_inc(pe1_sem, 1)
            # mm2: window pairs (2pp, 2pp+1) interleaved across PE row groups;
            # window pl -> bank pl (freed by gelu pl before act_sem >= 5+pp)
            tensor.wait_ge(wo_sem, 16)
            for pp in range(4):
                tensor.wait_ge(act_sem, 5 + pp)  # g tile pp complete
                for hh in range(8):
                    for pl in (2 * pp, 2 * pp + 1):
                        jp = 64 * (pl % 2)
                        mm = nc.tensor.matmul(
                            ps[:, pl, :],
                            lhsT=g_t[jp : jp + 64, pp, 128 * hh : 128 * hh + 128],
                            rhs=wo_sl(hh, jp),
                            start=(hh == 0),
                            stop=(hh == 7),
                            skip_group_check=True,
                        )
                        if hh == 7:
                            mm.then_inc(pe2_sem, 1)

        @block.scalar
        def _(scalar):
            # Pre-warm the gelu spline table during the input DMA.
            scalar.wait_ge(dve_sem, 1)
            nc.scalar.activation(
                scratch[:, 8:], scratch[:, :8], mybir.ActivationFunctionType.Gelu
            )
            for i in range(8):
                m, nch = i % 4, i // 4
                scalar.wait_ge(pe1_sem, i + 1)
                nc.scalar.activation(
                    g_t[:, m, 512 * nch : 512 * nch + 512],
                    ps[:, 4 * nch + m, :],
                    mybir.ActivationFunctionType.Gelu,
                ).then_inc(act_sem, 1)

        @block.vector
        def _(vector):
            nc.vector.memset(scratch[:, :8], 0.0).then_inc(dve_sem, 1)
            for pl in range(8):
                vector.wait_ge(pe2_sem, pl + 1)
                nc.vector.tensor_copy(out_t[:, pl, :], ps[:, pl, :]).then_inc(
                    dve_sem, 1
                )

    return nc


def _shard_inputs(x, W_qkv, W_out):
    bf16 = ml_dtypes.bfloat16
    W_v = np.ascontiguousarray(W_qkv[:, 2 * INNER : 3 * INNER]).astype(bf16)
    wv_chunks = W_v.reshape(4, 128, 512).transpose(1, 0, 2)  # (128, 4, 512)
    # wo[H] = W_out rows [64H, 64H+64) duplicated to both partition halves.
    wo = np.empty((8, 128, 512), dtype=bf16)
    for h in range(8):
        blk = W_out[64 * h : 64 * h + 64, :].astype(bf16)
        wo[h, :64] = blk
        wo[h, 64:] = blk
    wo_part = wo.transpose(1, 0, 2).reshape(128, 4096)
    in_maps = []
    for c in range(N_CORES):
        b, q, t = c // 4, (c // 2) % 2, c % 2
        xs = x[b, q::2, TOK_HALF * t : TOK_HALF * (t + 1), :]  # (8, 128, 512)
        xt = np.ascontiguousarray(xs.transpose(2, 0, 1).reshape(512, 1024)).astype(bf16)
        xt4 = xt.reshape(4, 128, 1024)
        xtA = xt4[:, :, :512].transpose(1, 0, 2)  # (128, 4, 512)
        xtB = xt4[:, :, 512:].transpose(1, 0, 2).reshape(128, 2048)
        front = np.concatenate([wv_chunks, xtA], axis=2).reshape(128, 4096)
        mega = np.concatenate([front, xtB, wo_part], axis=1)
        in_maps.append({"inp": np.ascontiguousarray(mega)})
    return in_maps


def _assemble(results, b_out):
    out = np.empty((B, P_WIN, N_TOK, DIM), dtype=np.float32)
    for c in range(N_CORES):
        b, q, t = c // 4, (c // 2) % 2, c % 2
        r = np.asarray(results[c]["out"]).astype(np.float32)  # (8, 128, 512)
        out[b, 8 * q : 8 * q + 8, TOK_HALF * t : TOK_HALF * (t + 1), :] = r
    out += b_out.astype(np.float32)
    return out


def _run(inputs, trace=False, trace_cores=None):
    global _COMPILED
    from concourse.bass_utils import run_bass_kernel_spmd

    if _COMPILED is None:
        _COMPILED = _build_bass()
    nc = _COMPILED
    in_maps = _shard_inputs(
        np.asarray(inputs["x"]), np.asarray(inputs["W_qkv"]), np.asarray(inputs["W_out"])
    )
    res = run_bass_kernel_spmd(
        nc, in_maps, core_ids=list(range(N_CORES)), trace=trace, trace_cores=trace_cores
    )
    out = _assemble(res.results, np.asarray(inputs["b_out"]))
    return out, res


def kernel(x, grad, W_qkv, W_out, b_out):
    out, _ = _run(dict(x=x, grad=grad, W_qkv=W_qkv, W_out=W_out, b_out=b_out))
    return out


# revision 15
# speedup vs baseline: 1.5969x; 1.0371x over previous
"""Trainium2 Bass kernel for nn_Attention_38130719654026 (sparse_attention).

The reference collapses exactly (verified to 9e-8 rel err): the top-k gather
broadcasts kv over the topk axis, so attention logits are constant along it,
softmax is uniform, and attn @ v_sel returns v unchanged.  grad/q/k are dead.
What remains is:

    g   = gelu(x @ W_v)              with W_v = W_qkv[:, 1024:1536]
    y[b, P, n, 64H:64H+64] = g[b, 2H + P//8, n, 64*(P%8) : 64*(P%8)+64]
    out = y @ W_out + b_out

Sharding (8 cores, no collectives): core c -> (batch b = c//4,
window parity q = (c//2)%2, token half t = c%2).  Each core computes the 8
output windows P in [8q, 8q+8) for its 128-token slice; these need exactly
the 8 source windows p = q + 2H at the same tokens, so x is read exactly
once across the fleet.

Raw bass (no TileContext: this walrus build rejects >1 embedded sync wait
per instruction, which Tile's kernel-tail drain always produces).  bf16
compute, fp32 PSUM accumulate, bf16 output (host casts back to fp32).

Pipeline per core (one PSUM bank per matmul group, all 8 banks used):
  7 input DMAs gate compute progressively: wv -> xtA k-chunks (k-major
  mm1 starts after the first 0.625 MB) -> xtB -> wo.  Gelu on ScalarE
  (spline table pre-warmed during the DMA), PSUM->SBUF copies on DVE,
  4 progressive output DMAs.  mm2 emits window pairs interleaved:
  adjacent matmuls use PE row groups 0-1 vs 2-3 (partition base 0 vs 64)
  and different PSUM banks, so they run concurrently.
"""

import sys

sys.path.insert(0, "/opt/trn_rl_repo")

import numpy as np
import ml_dtypes

B, P_WIN, N_TOK, DIM = 2, 16, 256, 512
H_HEADS, DH = 8, 64
INNER = H_HEADS * DH  # 512
TOK_HALF = N_TOK // 2  # 128
N_CORES = 8

# Column layout of the packed input mega-tile (128 partitions, bf16):
#   [1024k : 1024k+512)    wv k-chunk;  [1024k+512 : 1024k+1024) xtA k-chunk
#   [4096:6144)   xtB : X^T token-half B, 4 k-chunks of (128, 512)
#   [6144:10240)  wo  : W_out as 8 dup-row tiles of (128, 512)
XTB_OFF, WO_OFF, IN_COLS = 4096, 6144, 10240

_COMPILED = None


def _build_bass():
    import concourse.bass as bass
    import concourse.mybir as mybir

    dt = mybir.dt
    nc = bass.Bass()

    inp = nc.declare_dram_parameter("inp", [128, IN_COLS], dt.bfloat16, isOutput=False)
    out = nc.declare_dram_parameter("out", [8, 128, 512], dt.bfloat16, isOutput=True)

    with (
        nc.sbuf_tensor([128, IN_COLS], dt.bfloat16) as mega,
        nc.sbuf_tensor([128, 4, 1024], dt.bfloat16) as g_t,
        nc.sbuf_tensor([128, 8, 512], dt.bfloat16) as out_t,
        nc.sbuf_tensor([128, 16], dt.float32) as scratch,
        nc.sbuf_tensor([128, 192], dt.bfloat16) as scratch_bf,
        nc.psum_tensor([128, 8, 512], dt.float32) as ps,
        nc.semaphore("ka0_sem") as ka0_sem,
        nc.semaphore("ka1_sem") as ka1_sem,
        nc.semaphore("ka2_sem") as ka2_sem,
        nc.semaphore("ka3_sem") as ka3_sem,
        nc.semaphore("xb1_sem") as xb1_sem,
        nc.semaphore("xb2_sem") as xb2_sem,
        nc.semaphore("wo_sem") as wo_sem,
        nc.semaphore("dmao_sem") as dmao_sem,
        nc.semaphore("pe1_sem") as pe1_sem,
        nc.semaphore("pe2_sem") as pe2_sem,
        nc.semaphore("act_sem") as act_sem,
        nc.semaphore("dve_sem") as dve_sem,
        nc.Block() as block,
    ):
        ka_sems = [ka0_sem, ka1_sem, ka2_sem, ka3_sem]

        def xt_sl(k, nch):
            off = (1024 * k + 512) if nch == 0 else (XTB_OFF + 512 * k)
            return mega[:, off : off + 512]

        def wv_sl(k, m):
            return mega[:, 1024 * k + 128 * m : 1024 * k + 128 * m + 128]

        def wo_sl(h, jp):
            return mega[jp : jp + 64, WO_OFF + 512 * h : WO_OFF + 512 * h + 512]

        @block.sync
        def _(sync):
            for k in range(4):
                o = 1024 * k
                sync.dma_start(out=mega[:, o : o + 1024], in_=inp[:, o : o + 1024]).then_inc(
                    ka_sems[k], 16
                )
            mid = XTB_OFF + 1024
            sync.dma_start(out=mega[:, XTB_OFF:mid], in_=inp[:, XTB_OFF:mid]).then_inc(xb1_sem, 16)
            sync.dma_start(out=mega[:, mid:WO_OFF], in_=inp[:, mid:WO_OFF]).then_inc(xb2_sem, 16)
            sync.dma_start(out=mega[:, WO_OFF:], in_=inp[:, WO_OFF:]).then_inc(wo_sem, 16)
            for pl in range(8):
                sync.wait_ge(dve_sem, pl + 3)
                sync.dma_start(
                    out=out[pl].rearrange("p t -> p t"),
                    in_=out_t[:, pl, :],
                ).then_inc(dmao_sem, 16)
            sync.wait_ge(dmao_sem, 128)

        @block.tensor
        def _(tensor):
            # Warm the PE HAM clock with small dummy matmuls during the DMA.
            tensor.wait_ge(dve_sem, 2)
            for _ in range(36):
                nc.tensor.matmul(
                    ps[:, 0, 0:64],
                    lhsT=scratch_bf[:, 0:128],
                    rhs=scratch_bf[:, 128:192],
                    start=True,
                    stop=True,
                    skip_group_check=True,
                )
            # mm1 (k-major): gT = W_v^T @ X^T; group (m, nch) -> bank 4*nch+m
            for nch in range(2):
                for k in range(4):
                    if nch == 0:
                        tensor.wait_ge(ka_sems[k], 16)
                    elif k == 0:
                        tensor.wait_ge(xb1_sem, 16)
                    elif k == 2:
                        tensor.wait_ge(xb2_sem, 16)
                    for m in range(4):
                        mm = nc.tensor.matmul(
                            ps[:, 4 * nch + m, :],
                            lhsT=wv_sl(k, m),
                            rhs=xt_sl(k, nch),
                            start=(k == 0),
                            stop=(k == 3),
                            skip_group_check=True,
                        )
                        if k == 3:
                            mm.then_inc(pe1_sem, 1)
            # mm2: window pairs (2pp, 2pp+1) interleaved across PE row groups;
            # window pl -> bank pl (freed by gelu pl before act_sem >= 5+pp)
            tensor.wait_ge(wo_sem, 16)
            for pp in range(4):
                tensor.wait_ge(act_sem, 5 + pp)  # g tile pp complete
                for hh in range(8):
                    for pl in (2 * pp, 2 * pp + 1):
                        jp = 64 * (pl % 2)
                        mm = nc.tensor.matmul(
                            ps[:, pl, :],
                            lhsT=g_t[jp : jp + 64, pp, 128 * hh : 128 * hh + 128],
                            rhs=wo_sl(hh, jp),
                            start=(hh == 0),
                            stop=(hh == 7),
                            skip_group_check=True,
                        )
                        if hh == 7:
                            mm.then_inc(pe2_sem, 1)

        @block.scalar
        def _(scalar):
            # Pre-warm the gelu spline table during the input DMA.
            scalar.wait_ge(dve_sem, 1)
            nc.scalar.activation(
                scratch[:, 8:], scratch[:, :8], mybir.ActivationFunctionType.Gelu
            )
            for i in range(8):
                m, nch = i % 4, i // 4
                scalar.wait_ge(pe1_sem, i + 1)
                nc.scalar.activation(
                    g_t[:, m, 512 * nch : 512 * nch + 512],
                    ps[:, 4 * nch + m, :],
                    mybir.ActivationFunctionType.Gelu,
                ).then_inc(act_sem, 1)

        @block.vector
        def _(vector):
            nc.vector.memset(scratch[:, :8], 0.0).then_inc(dve_sem, 1)
            nc.vector.memset(scratch_bf[:], 0.0).then_inc(dve_sem, 1)
            for pl in range(8):
                vector.wait_ge(pe2_sem, pl + 1)
                nc.vector.tensor_copy(out_t[:, pl, :], ps[:, pl, :]).then_inc(
                    dve_sem, 1
                )

    return nc


def _shard_inputs(x, W_qkv, W_out):
    bf16 = ml_dtypes.bfloat16
    W_v = np.ascontiguousarray(W_qkv[:, 2 * INNER : 3 * INNER]).astype(bf16)
    wv_chunks = W_v.reshape(4, 128, 512).transpose(1, 0, 2)  # (128, 4, 512)
    # wo[H] = W_out rows [64H, 64H+64) duplicated to both partition halves.
    wo = np.empty((8, 128, 512), dtype=bf16)
    for h in range(8):
        blk = W_out[64 * h : 64 * h + 64, :].astype(bf16)
        wo[h, :64] = blk
        wo[h, 64:] = blk
    wo_part = wo.transpose(1, 0, 2).reshape(128, 4096)
    in_maps = []
    for c in range(N_CORES):
        b, q, t = c // 4, (c // 2) % 2, c % 2
        xs = x[b, q::2, TOK_HALF * t : TOK_HALF * (t + 1), :]  # (8, 128, 512)
        xt = np.ascontiguousarray(xs.transpose(2, 0, 1).reshape(512, 1024)).astype(bf16)
        xt4 = xt.reshape(4, 128, 1024)
        xtA = xt4[:, :, :512].transpose(1, 0, 2)  # (128, 4, 512)
        xtB = xt4[:, :, 512:].transpose(1, 0, 2).reshape(128, 2048)
        front = np.concatenate([wv_chunks, xtA], axis=2).reshape(128, 4096)
        mega = np.concatenate([front, xtB, wo_part], axis=1)
        in_maps.append({"inp": np.ascontiguousarray(mega)})
    return in_maps


def _assemble(results, b_out):
    out = np.empty((B, P_WIN, N_TOK, DIM), dtype=np.float32)
    for c in range(N_CORES):
        b, q, t = c // 4, (c // 2) % 2, c % 2
        r = np.asarray(results[c]["out"]).astype(np.float32)  # (8, 128, 512)
        out[b, 8 * q : 8 * q + 8, TOK_HALF * t : TOK_HALF * (t + 1), :] = r
    out += b_out.astype(np.float32)
    return out


def _run(inputs, trace=False, trace_cores=None):
    global _COMPILED
    from concourse.bass_utils import run_bass_kernel_spmd

    if _COMPILED is None:
        _COMPILED = _build_bass()
    nc = _COMPILED
    in_maps = _shard_inputs(
        np.asarray(inputs["x"]), np.asarray(inputs["W_qkv"]), np.asarray(inputs["W_out"])
    )
    res = run_bass_kernel_spmd(
        nc, in_maps, core_ids=list(range(N_CORES)), trace=trace, trace_cores=trace_cores
    )
    out = _assemble(res.results, np.asarray(inputs["b_out"]))
    return out, res


def kernel(x, grad, W_qkv, W_out, b_out):
    out, _ = _run(dict(x=x, grad=grad, W_qkv=W_qkv, W_out=W_out, b_out=b_out))
    return out
